# revision 106
# baseline (speedup 1.0000x reference)
"""Causal self-attention (B=4, T=2048, C=1024, H=16) on 8 trn2 NeuronCores.

Sharding: 8 shards = (batch b in 0..3) x (half-of-heads hh in 0..1).
Each core computes, for its batch b and its 8 heads:
  - Q/K/V projections (bf16 matmuls, fp32 accum), with Q^T/K^T produced in
    transposed [c_out, t] layout so attention needs no on-device transposes
  - scores^T[k, q] = K_h^T.T @ Q_h^T (two heads packed in the 128-partition
    dim via 64-row PE array tiling), exp on ACT (unnormalized softmax),
    causal mask via precomputed multiplicative mask tiles on DVE
  - AV with the 128-q-row block of expS as the STATIONARY operand and
    V' (V plus a ones-column) as the 65-wide MOVING operand: each matmul
    streams only 65 columns instead of 512, halving AV's PE time vs the
    O^T = V'^T @ expS formulation (PE cost is proportional to the moving
    free size only). Output O[q, d] lands q-major with the softmax
    denominator Z in column 64; two q-subtiles (x 2 heads x 65) share one
    PSUM bank, so all four subtile accumulations of a (chunk, pair) unit
    are live at once.
  - The QK/exp/AV pipeline is k-tile-major with a LAG-tile lag: each step
    runs QK(kt) and the AV matmuls of kt-LAG across all valid q-subtiles,
    pacing the PE naturally just under the ACT exp chain (~930ns/tile)
    with ~1-2 spliced filler matmuls of slack per tile, and releasing
    es[kt] early for the next unit's exp (WAR).
  - normalize with a per-partition 1/Z (DVE reciprocal + one broadcast
    tensor-tensor mul into a [q, 128] head-pair tile); one PE transpose
    per head-pair per q-subtile back to [c_in, t] layout + DVE copy into
    ON. All four transposes of a unit are deferred into the next unit's
    early QK phase, which both covers the normalize latency and makes the
    2-slot PSUM ring reuse each bank only after its readers finished.
  - out-projection y_part = ON.T @ Wo[hh-slice, :], interleaved as filler.
Chunks are processed in order 1, 3, 2, 0: the first overlaps the input
DMA stream, the exp-heaviest chunks run mid-kernel where mandatory
projection work doubles as PE filler for the ACT-bound exp chains, and
the kernel ends on chunk 0 whose exp chains are tiny (short tail, with
the final tiles' out-projections emitted per subtile). Host sums the two
partials per batch and adds bo (with bv folded through Wo).
"""

import os
import sys

for _p in ("/opt/trn_rl_repo",):
    if _p not in sys.path and os.path.isdir(_p):
        sys.path.insert(0, _p)

from contextlib import ExitStack

import ml_dtypes
import numpy as np

import concourse.bacc as bacc
import concourse.mybir as mybir
import concourse.tile as tile
from concourse import bass_utils

B, T, C, H = 4, 2048, 1024, 16
D = 64                 # head dim
HLOC = 8               # heads per core
CS = HLOC * D          # 512: per-core slice of C on the head axis
SCALE = 1.0 / 8.0      # 1/sqrt(D)
NP = 128               # partitions
QC = 512               # q chunk (PSUM bank width in fp32)
NQC = T // QC          # 4
NKT = T // NP          # 16 k tiles
NCT = C // NP          # 8 contraction tiles for projections
NPAIR = HLOC // 2      # 4 head pairs

F32 = mybir.dt.float32
BF16 = mybir.dt.bfloat16
BF = ml_dtypes.bfloat16

_CACHE = {}


def _build():
    nc = bacc.Bacc(
        "TRN2",
        target_bir_lowering=False,
        debug=False,
        enable_asserts=False,
        num_devices=8,
    )

    qT_d = nc.dram_tensor("qT", [C, T], BF16, kind="ExternalInput").ap()
    kT_d = nc.dram_tensor("kT", [C, T], BF16, kind="ExternalInput").ap()
    vT_d = nc.dram_tensor("vT", [C, T], BF16, kind="ExternalInput").ap()
    Wq_d = nc.dram_tensor("Wq", [C, CS], BF16, kind="ExternalInput").ap()
    Wk_d = nc.dram_tensor("Wk", [C, CS], BF16, kind="ExternalInput").ap()
    Wv_d = nc.dram_tensor("Wv", [C, CS], BF16, kind="ExternalInput").ap()
    Wo_d = nc.dram_tensor("Wo", [CS, C], BF16, kind="ExternalInput").ap()
    bq_d = nc.dram_tensor("bq", [CS], F32, kind="ExternalInput").ap()
    bk_d = nc.dram_tensor("bk", [CS], F32, kind="ExternalInput").ap()
    y_d = nc.dram_tensor("y", [T, C], F32, kind="ExternalOutput").ap()

    with tile.TileContext(nc) as tc, ExitStack() as ctx:
        wpool = ctx.enter_context(tc.tile_pool(name="wpool", bufs=1))
        cpool = ctx.enter_context(tc.tile_pool(name="cpool", bufs=1))
        xpool = ctx.enter_context(tc.tile_pool(name="xpool", bufs=2))
        epool = ctx.enter_context(tc.tile_pool(name="epool", bufs=1))
        spool = ctx.enter_context(tc.tile_pool(name="spool", bufs=3))
        ypool = ctx.enter_context(tc.tile_pool(name="ypool", bufs=3))
        psP = ctx.enter_context(tc.tile_pool(name="psP", bufs=2, space="PSUM"))
        psS = ctx.enter_context(tc.tile_pool(name="psS", bufs=2, space="PSUM"))
        psO = ctx.enter_context(tc.tile_pool(name="psO", bufs=2, space="PSUM"))

        # ---- persistent weights / consts ----
        # Weights live in single wide tiles. DMA priority order: Wq + qT
        # first-chunk quarters first (interleaved across the SP and ACT
        # queues) so the Q projection can start after ~256KB instead of ~1MB.
        def w_tile_and_dma(name, dram, n_ct, width, dt):
            t = wpool.tile([NP, n_ct * width], dt, name=name, tag=name)
            nc.sync.dma_start(
                t.rearrange("p (j n) -> p j n", n=width),
                dram.rearrange("(j p) n -> p j n", p=NP),
            )
            return t

        xcur = {}

        def emit_x_dma(c, inputs=(0, 1, 2)):
            for ii in inputs:
                x_d = (qT_d, kT_d, vT_d)[ii]
                xt = xpool.tile([NP, NCT * QC], BF16, name=f"x{ii}", tag=f"x{ii}")
                nc.sync.dma_start(
                    xt.rearrange("p (j n) -> p j n", n=QC),
                    x_d[:, c * QC : (c + 1) * QC].rearrange("(j p) n -> p j n", p=NP),
                )
                xcur[(c, ii)] = xt

        def w_part_dma(t, dram, width, i, parts):
            h = NCT // parts
            nc.sync.dma_start(
                t[:, i * h * width : (i + 1) * h * width].rearrange(
                    "p (j n) -> p j n", n=width
                ),
                dram[i * h * NP : (i + 1) * h * NP, :].rearrange(
                    "(j p) n -> p j n", p=NP
                ),
            )

        def x_part_dma(c, ii, i, parts):
            x_d = (qT_d, kT_d, vT_d)[ii]
            h = NCT // parts
            nc.scalar.dma_start(
                xcur[(c, ii)][:, i * h * QC : (i + 1) * h * QC].rearrange(
                    "p (j n) -> p j n", n=QC
                ),
                x_d[i * h * NP : (i + 1) * h * NP, c * QC : (c + 1) * QC]
                .rearrange("(j p) n -> p j n", p=NP),
            )

        # Attention chunks are processed in order 1, 3, 2, 0: the first chunk
        # is medium-cost (overlaps the input DMA stream), the exp-heaviest
        # chunks run mid-kernel where mandatory projection work doubles as
        # PE filler for the ACT-bound exp chains, and the kernel ends on
        # chunk 0 whose exp chains are tiny (short tail).
        CORDER = [1, 3, 2, 0]
        C0 = CORDER[0]

        Wq_sb = wpool.tile([NP, NCT * CS], BF16, name="Wq", tag="Wq")
        Wk_sb = wpool.tile([NP, NCT * CS], BF16, name="Wk", tag="Wk")
        xcur[(C0, 0)] = xpool.tile([NP, NCT * QC], BF16, name="x0", tag="x0")
        xcur[(C0, 1)] = xpool.tile([NP, NCT * QC], BF16, name="x1", tag="x1")
        bq_sb = cpool.tile([NP, 4], F32, name="bq_sb", tag="bq_sb")
        bk_sb = cpool.tile([NP, 4], F32, name="bk_sb", tag="bk_sb")
        # Wq/x in quarters, interleaved: first matmul can start after one
        # quarter of each has landed (~2 x 256KB).
        for i in range(4):
            w_part_dma(Wq_sb, Wq_d, CS, i, 4)
            x_part_dma(C0, 0, i, 4)
            if i == 0:
                nc.sync.dma_start(bq_sb[:], bq_d.rearrange("(t p) -> p t", p=NP))
        for i in range(2):
            w_part_dma(Wk_sb, Wk_d, CS, i, 2)
            x_part_dma(C0, 1, i, 2)
            if i == 0:
                nc.sync.dma_start(bk_sb[:], bk_d.rearrange("(t p) -> p t", p=NP))
        Wv_sb = w_tile_and_dma("Wv", Wv_d, NCT, CS, BF16)
        emit_x_dma(0, inputs=(2,))
        emit_x_dma(1, inputs=(2,))
        Wo_sb = w_tile_and_dma("Wo", Wo_d, NPAIR, C, BF16)

        # Causal mask for the single partially-masked [128,128] block of each
        # diagonal tile. Duplicated in two halves so both heads mask with one
        # op: mask[p, f%128] = 1.0 if f%128 >= p.
        mask_sb = cpool.tile([NP, 2 * NP], BF16, name="mask_sb", tag="mask_sb")
        nc.gpsimd.memset(mask_sb[:], 1.0)
        nc.gpsimd.affine_select(
            out=mask_sb.rearrange("p (h f) -> p h f", h=2),
            in_=mask_sb.rearrange("p (h f) -> p h f", h=2),
            pattern=[[0, 2], [1, NP]],
            compare_op=mybir.AluOpType.is_ge,
            fill=0.0,
            base=0,
            channel_multiplier=-1,
        )

        # identity (permutation) matrix for PE transposes
        ident_sb = cpool.tile([NP, NP], BF16, name="ident_sb", tag="ident_sb")
        nc.gpsimd.memset(ident_sb[:], 1.0)
        nc.gpsimd.affine_select(
            out=ident_sb[:],
            in_=ident_sb[:],
            pattern=[[1, NP]],
            compare_op=mybir.AluOpType.is_equal,
            fill=0.0,
            base=0,
            channel_multiplier=-1,
        )

        # persistent activations
        QT_sb = [
            cpool.tile([NP, T], BF16, name=f"QT{p}", tag=f"QT{p}") for p in range(NPAIR)
        ]
        KT_sb = [
            cpool.tile([NP, T], BF16, name=f"KT{p}", tag=f"KT{p}") for p in range(NPAIR)
        ]
        ON_sb = [
            cpool.tile([NP, T], BF16, name=f"ON{p}", tag=f"ON{p}") for p in range(NPAIR)
        ]
        # V' padded: per k-tile [128, 8 heads * 65], col 64 of each 65-block = 1.0
        V_sb = [
            cpool.tile([NP, HLOC * 65], BF16, name=f"V{t}", tag=f"V{t}")
            for t in range(NKT)
        ]
        for t in range(NKT):
            v3 = V_sb[t].rearrange("p (h e) -> p h e", e=65)
            nc.any.memset(v3[:, :, 64:65], 1.0)

        # ---------------- emission helpers ----------------
        def gen_qk_proj_first(W_sb, b_sb, OUT, ii, ots, qc):
            """First-chunk Q/K projection for an ot pair, j-major in quarters
            to match the quarter-DMA arrival order."""
            pss = {
                ot: psP.tile([NP, QC], F32, name="ps_proj", tag="ps_proj")
                for ot in ots
            }
            for jq in range(4):
                for ot in ots:
                    for j in (jq * 2, jq * 2 + 1):
                        last = j == NCT - 1

                        def mm(ps=pss[ot], ot=ot, j=j, last=last):
                            nc.tensor.matmul(
                                ps[:],
                                W_sb[:, j * CS + ot * NP : j * CS + (ot + 1) * NP],
                                xcur[(qc, ii)][:, j * QC : (j + 1) * QC],
                                start=(j == 0),
                                stop=last,
                            )
                            if last:
                                nc.vector.tensor_scalar_add(
                                    OUT[ot][:, qc * QC : (qc + 1) * QC],
                                    ps[:],
                                    b_sb[:, ot : ot + 1],
                                )

                        yield mm

        def gen_qk_proj_part(qc, ot):
            """Q^T and K^T projection matmuls for chunk qc, c_out tile ot.
            Chunk 0 is processed last, where ACT is idle and DVE-queue
            latency gates the next QK matmuls — evacuate via ACT there."""
            for ii, (W_sb, b_sb, OUT) in enumerate(
                ((Wq_sb, bq_sb, QT_sb), (Wk_sb, bk_sb, KT_sb))
            ):
                ps = psP.tile([NP, QC], F32, name="ps_proj", tag="ps_proj")
                for j in range(NCT):
                    last = j == NCT - 1

                    def mm(ps=ps, ii=ii, W_sb=W_sb, b_sb=b_sb, OUT=OUT, j=j, last=last):
                        nc.tensor.matmul(
                            ps[:],
                            W_sb[:, j * CS + ot * NP : j * CS + (ot + 1) * NP],
                            xcur[(qc, ii)][:, j * QC : (j + 1) * QC],
                            start=(j == 0),
                            stop=last,
                        )
                        if last:
                            nc.vector.tensor_scalar_add(
                                OUT[ot][:, qc * QC : (qc + 1) * QC],
                                ps[:],
                                b_sb[:, ot : ot + 1],
                            )

                    yield mm

        def gen_v_proj_part(qc, ts):
            """V projection matmuls for chunk qc, t-subtile ts."""
            t_tile = qc * 4 + ts
            ps = psP.tile([NP, QC], F32, name="ps_proj", tag="ps_proj")
            for j in range(NCT):
                last = j == NCT - 1

                def mm(ps=ps, j=j, last=last, t_tile=t_tile):
                    nc.tensor.matmul(
                        ps[:],
                        xcur[(qc, 2)][:, j * QC + ts * NP : j * QC + (ts + 1) * NP],
                        Wv_sb[:, j * CS : (j + 1) * CS],
                        start=(j == 0),
                        stop=last,
                    )
                    if last:
                        dst = V_sb[t_tile].rearrange("p (h e) -> p h e", e=65)[
                            :, :, 0:64
                        ]
                        src = ps.rearrange("p (h d) -> p h d", d=D)
                        nc.vector.tensor_copy(dst, src)

                yield mm

        def gen_out_proj_part(tc_, p, act_evac=False):
            """Out-projection for t_tile 4*tc_+p, both 512-wide n chunks.
            act_evac: evacuate via ACT — for groups consumed at the tail,
            where ACT is idle but the DVE queue (normalizes, ON copies)
            gates the PE."""
            tt = 4 * tc_ + p
            tsl = slice(tt * NP, (tt + 1) * NP)
            ysb = ypool.tile([NP, C], F32, name="ysb", tag="ysb")
            for nck in range(2):
                nsl = slice(nck * QC, (nck + 1) * QC)
                ps = psP.tile([NP, QC], F32, name="ps_proj", tag="ps_proj")
                for pair in range(NPAIR):
                    last = pair == NPAIR - 1

                    def mm(ps=ps, pair=pair, last=last, tsl=tsl, nsl=nsl, nck=nck):
                        nc.tensor.matmul(
                            ps[:],
                            ON_sb[pair][:, tsl],
                            Wo_sb[:, pair * C + nsl.start : pair * C + nsl.stop],
                            start=(pair == 0),
                            stop=last,
                        )
                        if last:
                            if act_evac:
                                nc.scalar.copy(ysb[:, nsl], ps[:])
                            else:
                                nc.vector.tensor_copy(ysb[:, nsl], ps[:])
                            if nck == 1:
                                nc.sync.dma_start(y_d[tsl, :], ysb[:])

                    yield mm

        def emit_outproj_tile(tt, nw=QC):
            """Inline out-projection of one t-tile in n-chunks of nw, heads
            pair 3 last (its ON slice is the freshest), DVE-side evacuation
            (ACT is exp-saturated in the late chunks), DMA per n-chunk."""
            tsl = slice(tt * NP, (tt + 1) * NP)
            ysb = ypool.tile([NP, C], F32, name="ysb", tag="ysb")
            for nck in range(C // nw):
                nsl = slice(nck * nw, (nck + 1) * nw)
                ps = psP.tile([NP, nw], F32, name="ps_proj", tag="ps_proj")
                for pair in range(NPAIR):
                    nc.tensor.matmul(
                        ps[:],
                        ON_sb[pair][:, tsl],
                        Wo_sb[:, pair * C + nsl.start : pair * C + nsl.stop],
                        start=(pair == 0),
                        stop=(pair == NPAIR - 1),
                    )
                nc.vector.tensor_copy(ysb[:, nsl], ps[:])
                # alternate DMA queues so the final transfers don't queue
                # behind earlier y DMAs on the in-order SP sequencer
                eng = nc.scalar if nck % 2 else nc.sync
                eng.dma_start(y_d[tsl, nsl], ysb[:, nsl])

        # ---------------- filler queues ----------------
        # items: (attention-order index, closure); the index caps front-running
        proj_q = []
        mark_qk = {}   # (qc, pair) -> proj_q index that must be drained first
        mark_av = {}   # qc -> proj_q index that must be drained before AV

        def x_dma_item(c, inputs):
            def f(c=c, inputs=tuple(inputs)):
                emit_x_dma(c, inputs=inputs)

            return f

        # first chunk (C0): Q(ot 0,1), K(ot 0,1) -> pairs 0,1 ready early.
        for ots in ((0, 1), (2, 3)):
            for ii, W_sb, b_sb, OUT in (
                (0, Wq_sb, bq_sb, QT_sb),
                (1, Wk_sb, bk_sb, KT_sb),
            ):
                proj_q.extend(
                    (0, f)
                    for f in gen_qk_proj_first(W_sb, b_sb, OUT, ii, ots, C0)
                )
            mark_qk[(C0, ots[0])] = mark_qk[(C0, ots[1])] = len(proj_q)
        # V projections for chunks 0,1 (the first attention chunk needs both);
        # per-tile marks let the AV steps drain V just-in-time
        mark_v = {}
        for c in (0, 1):
            for ts in range(NPAIR):
                proj_q.extend((0, f) for f in gen_v_proj_part(c, ts))
                mark_v[4 * c + ts] = len(proj_q)

        # chunk 3 Q/K (attention order 1), with V for chunks 2,3 inside
        proj_q.append((1, x_dma_item(3, (0, 1))))
        for p in range(NPAIR):
            proj_q.extend((1, f) for f in gen_qk_proj_part(3, p))
            mark_qk[(3, p)] = len(proj_q)
            if p == 0:
                proj_q.append((1, x_dma_item(2, (2,))))
                for ts in range(NPAIR):
                    proj_q.extend((1, f) for f in gen_v_proj_part(2, ts))
                    mark_v[8 + ts] = len(proj_q)
                proj_q.append((1, x_dma_item(3, (2,))))
                for ts in range(NPAIR):
                    proj_q.extend((1, f) for f in gen_v_proj_part(3, ts))
                    mark_v[12 + ts] = len(proj_q)
        # chunk 2 Q/K (attention order 2)
        proj_q.append((2, x_dma_item(2, (0, 1))))
        for p in range(NPAIR):
            proj_q.extend((2, f) for f in gen_qk_proj_part(2, p))
            mark_qk[(2, p)] = len(proj_q)
        # chunk 0 Q/K (attention order 3)
        proj_q.append((3, x_dma_item(0, (0, 1))))
        for p in range(NPAIR):
            proj_q.extend((3, f) for f in gen_qk_proj_part(0, p))
            mark_qk[(0, p)] = len(proj_q)

        op_q = []      # eligible out-proj closures (appended as chunks finish)

        state = {"pq": 0, "qc": 0}

        def drain_to(idx):
            while state["pq"] < idx:
                proj_q[state["pq"]][1]()
                state["pq"] += 1

        def splice(n):
            # pop projection filler, but never front-run more than one chunk
            # ahead of the current attention chunk
            k = 0
            while (
                k < n
                and state["pq"] < len(proj_q)
                and proj_q[state["pq"]][0] <= state["qc"] + 1
            ):
                proj_q[state["pq"]][1]()
                state["pq"] += 1
                k += 1
            if k == 0 and op_q:
                # ration out-proj filler so it lasts through the final chunk
                state["tick"] = state.get("tick", 0) + 1
                if state["tick"] % 2 == 0 or state["qc"] >= NQC - 2:
                    op_q.pop(0)()

        # ---------------- attention with interleaved filler ----------------
        for oi, qc in enumerate(CORDER):
            state["qc"] = oi
            kmax = 4 * (qc + 1)
            for pair in range(NPAIR):
                last_sec = oi == NQC - 1 and pair == NPAIR - 1
                drain_to(mark_qk[(qc, pair)])
                if oi == NQC - 1:
                    # the final chunk's units are short: force the excess
                    # out-proj backlog through now so none trails the last
                    # tile (and so it covers the proj bias-add latency that
                    # gates this unit's first QK matmuls)
                    while len(op_q) > (NPAIR - 1 - pair) * 12 + 8:
                        op_q.pop(0)()
                es = []
                # AV accumulators: two q-subtiles share one PSUM bank
                # (2 heads x 65 cols each), so all four qsub accumulations
                # are live at once and the AV matmuls can interleave with
                # the QK/exp pipeline k-tile-major.
                banks = [
                    psO.tile([NP, 260], F32, name="O2a", tag="O"),
                    psO.tile([NP, 260], F32, name="O2b", tag="O"),
                ]
                pairTs = []

                def norm(qs, banks=banks, pairTs=pairTs):
                    O3 = banks[qs // 2][
                        :, (qs % 2) * 130 : (qs % 2) * 130 + 130
                    ].rearrange("p (h e) -> p h e", e=65)
                    zinv = spool.tile([NP, 2], F32, name="zinv", tag="zinv")
                    nc.vector.reciprocal(zinv[:], O3[:, :, 64:65])
                    pairT = spool.tile(
                        [NP, NP], BF16, name="pairT", tag="pairT", bufs=6
                    )
                    nc.vector.tensor_mul(
                        pairT.rearrange("p (a d) -> p a d", d=64),
                        O3[:, :, 0:64],
                        zinv[:]
                        .rearrange("p (a o) -> p a o", o=1)
                        .to_broadcast((NP, 2, 64)),
                    )
                    pairTs.append(pairT)

                def tr_copy(qs, pair=pair, qc=qc, pairTs=pairTs):
                    trp = psO.tile([NP, NP], BF16, name="trp", tag="O")
                    nc.tensor.transpose(trp[:], pairTs[qs][:], ident_sb[:])
                    tt = 4 * qc + qs
                    nc.vector.tensor_copy(
                        ON_sb[pair][:, tt * NP : (tt + 1) * NP], trp[:]
                    )

                # Merged QK/exp/AV pipeline, k-tile-major with a 2-tile lag:
                # each step runs QK(kt) and the AV matmuls of kt-LAG across
                # all valid q-subtiles, keeping the PE naturally paced just
                # under the ACT exp chain (~930ns/tile) with only ~1 filler
                # matmul of slack per tile. es[kt] is fully consumed at step
                # kt+LAG, releasing it early for the next unit's exp.
                LAG = 4
                for step in range(kmax + LAG):
                    if step < kmax:
                        kt = step
                        # diagonal tiles (kt >= 4*qc) only need the q-suffix
                        # [off, 512): columns below are fully causal-masked
                        off = max(0, (kt - 4 * qc) * NP)
                        ksl = slice(kt * NP, (kt + 1) * NP)
                        S2 = psS.tile([NP, 2 * QC], F32, name="S2", tag="S2")
                        for hp in range(2):
                            psl = slice(hp * 64, (hp + 1) * 64)
                            nc.tensor.matmul(
                                S2[:, hp * QC + off : (hp + 1) * QC],
                                KT_sb[pair][psl, ksl],
                                QT_sb[pair][psl, qc * QC + off : (qc + 1) * QC],
                                start=True,
                                stop=True,
                                tile_position=(hp * 64, 0),
                            )
                        e2 = epool.tile(
                            [NP, 2 * QC], BF16, name=f"e{kt}", tag=f"e{kt}"
                        )
                        s3 = S2.rearrange("p (h f) -> p h f", h=2)[:, :, off:]
                        e3 = e2.rearrange("p (h f) -> p h f", h=2)[:, :, off:]
                        nc.scalar.activation(
                            e3, s3, mybir.ActivationFunctionType.Exp, scale=SCALE
                        )
                        if off or kt == 4 * qc:  # diagonal: mask partial block
                            eb = e2.rearrange("p (h f) -> p h f", h=2)[
                                :, :, off : off + NP
                            ]
                            nc.vector.tensor_mul(
                                eb, eb, mask_sb.rearrange("p (h f) -> p h f", h=2)
                            )
                        es.append(e2)
                        # deferred transposes+copies of the previous unit's
                        # subtiles: the last normalize's latency is covered
                        # by this unit's early QK phase
                        if kt == 3 and state.get("pending") is not None:
                            state["pending"]()
                            state["pending"] = None
                    kt2 = step - LAG
                    if 0 <= kt2 < kmax:
                        drain_to(mark_v[kt2])
                        for qs in range(max(0, kt2 - 4 * qc), 4):
                            bank = banks[qs // 2]
                            base = (qs % 2) * 130
                            for hp in range(2):
                                h = pair * 2 + hp
                                nc.tensor.matmul(
                                    bank[
                                        :, base + hp * 65 : base + (hp + 1) * 65
                                    ],
                                    es[kt2][
                                        :,
                                        hp * QC + qs * NP : hp * QC
                                        + (qs + 1) * NP,
                                    ],
                                    V_sb[kt2][:, h * 65 : (h + 1) * 65],
                                    start=(
                                        kt2 == 0 and hp == 0 and qs % 2 == 0
                                    ),
                                    stop=(
                                        hp == 1
                                        and qs % 2 == 1
                                        and kt2 == 4 * qc + qs
                                    ),
                                )
                        # a q-subtile's chain completes at its diagonal tile
                        if kt2 >= 4 * qc:
                            norm(kt2 - 4 * qc)
                    # early steps carry no AV matmuls yet: room for one more
                    splice(2 if step < LAG + 3 else 1)
                if last_sec:
                    # transposes inline, interleaved with the final tiles'
                    # out-projections (which double as normalize-latency
                    # cover); tile 12+qs is ready right after tr_copy(qs)
                    for qs in range(4):
                        tr_copy(qs)
                        emit_outproj_tile(
                            4 * qc + qs, nw=QC if qs < 3 else QC // 2
                        )
                else:
                    # all four transposes deferred into the next unit: the
                    # psO ring then reuses each bank only after both its
                    # subtiles' normalizes have long completed
                    def pending(pair=pair, qc=qc, tr_copy=tr_copy):
                        for qs in range(4):
                            tr_copy(qs)
                        if pair == NPAIR - 1:
                            # chunk complete: its out-proj becomes eligible.
                            # Chunk 2's groups are consumed at the tail where
                            # ACT is idle and the DVE queue gates the PE.
                            for p in range(NPAIR):
                                op_q.extend(
                                    gen_out_proj_part(qc, p, act_evac=qc == 2)
                                )

                    state["pending"] = pending

        # ---------------- epilogue (normally everything is drained) --------
        drain_to(len(proj_q))
        while op_q:
            op_q.pop(0)()

    nc.compile()
    return nc


def get_nc():
    if "nc" not in _CACHE:
        _CACHE["nc"] = _build()
    return _CACHE["nc"]


def make_in_maps(k, v, q, Wq, bq, Wk, bk, Wv, bv, Wo, bo):
    k = np.asarray(k, np.float32)
    v = np.asarray(v, np.float32)
    q = np.asarray(q, np.float32)
    Wq = np.asarray(Wq, np.float32).astype(BF)
    Wk = np.asarray(Wk, np.float32).astype(BF)
    Wv = np.asarray(Wv, np.float32).astype(BF)
    Wo = np.asarray(Wo, np.float32).astype(BF)
    bq = np.asarray(bq, np.float32)
    bk = np.asarray(bk, np.float32)

    in_maps = []
    for core in range(8):
        b, hh = core // 2, core % 2
        sl = slice(hh * CS, (hh + 1) * CS)
        in_maps.append(
            {
                "qT": np.ascontiguousarray(q[b].T.astype(BF)),
                "kT": np.ascontiguousarray(k[b].T.astype(BF)),
                "vT": np.ascontiguousarray(v[b].T.astype(BF)),
                "Wq": np.ascontiguousarray(Wq[:, sl]),
                "Wk": np.ascontiguousarray(Wk[:, sl]),
                "Wv": np.ascontiguousarray(Wv[:, sl]),
                "Wo": np.ascontiguousarray(Wo[sl, :]),
                "bq": np.ascontiguousarray(bq[sl]),
                "bk": np.ascontiguousarray(bk[sl]),
            }
        )
    return in_maps


def kernel(k, v, q, Wq, bq, Wk, bk, Wv, bv, Wo, bo):
    nc = get_nc()
    in_maps = make_in_maps(k, v, q, Wq, bq, Wk, bk, Wv, bv, Wo, bo)
    res = bass_utils.run_bass_kernel_spmd(nc, in_maps, core_ids=list(range(8)))
    # softmax rows sum to 1, so the V bias passes through attention as a
    # constant: y += bv @ Wo. Fold it into the host-side bias add.
    bias = np.asarray(bo, np.float32) + np.asarray(bv, np.float32) @ np.asarray(
        Wo, np.float32
    )
    out = np.empty((B, T, C), np.float32)
    for b in range(B):
        out[b] = res.results[2 * b]["y"] + res.results[2 * b + 1]["y"] + bias
    return out


# revision 109
# speedup vs baseline: 1.0025x; 1.0025x over previous
"""Causal self-attention (B=4, T=2048, C=1024, H=16) on 8 trn2 NeuronCores.

Sharding: 8 shards = (batch b in 0..3) x (half-of-heads hh in 0..1).
Each core computes, for its batch b and its 8 heads:
  - Q/K/V projections (bf16 matmuls, fp32 accum), with Q^T/K^T produced in
    transposed [c_out, t] layout so attention needs no on-device transposes
  - scores^T[k, q] = K_h^T.T @ Q_h^T (two heads packed in the 128-partition
    dim via 64-row PE array tiling), exp on ACT (unnormalized softmax),
    causal mask via precomputed multiplicative mask tiles on DVE
  - AV with the 128-q-row block of expS as the STATIONARY operand and
    V' (V plus a ones-column) as the 65-wide MOVING operand: each matmul
    streams only 65 columns instead of 512, halving AV's PE time vs the
    O^T = V'^T @ expS formulation (PE cost is proportional to the moving
    free size only). Output O[q, d] lands q-major with the softmax
    denominator Z in column 64; two q-subtiles (x 2 heads x 65) share one
    PSUM bank, so all four subtile accumulations of a (chunk, pair) unit
    are live at once.
  - The QK/exp/AV pipeline is k-tile-major with a LAG-tile lag: each step
    runs QK(kt) and the AV matmuls of kt-LAG across all valid q-subtiles,
    pacing the PE naturally just under the ACT exp chain (~930ns/tile)
    with ~1-2 spliced filler matmuls of slack per tile, and releasing
    es[kt] early for the next unit's exp (WAR).
  - normalize with a per-partition 1/Z (DVE reciprocal + one broadcast
    tensor-tensor mul into a [q, 128] head-pair tile); one PE transpose
    per head-pair per q-subtile back to [c_in, t] layout + DVE copy into
    ON. All four transposes of a unit are deferred into the next unit's
    early QK phase, which both covers the normalize latency and makes the
    2-slot PSUM ring reuse each bank only after its readers finished.
  - out-projection y_part = ON.T @ Wo[hh-slice, :], interleaved as filler.
Chunks are processed in order 1, 3, 2, 0: the first overlaps the input
DMA stream, the exp-heaviest chunks run mid-kernel where mandatory
projection work doubles as PE filler for the ACT-bound exp chains, and
the kernel ends on chunk 0 whose exp chains are tiny (short tail, with
the final tiles' out-projections emitted per subtile). Host sums the two
partials per batch and adds bo (with bv folded through Wo).
"""

import os
import sys

for _p in ("/opt/trn_rl_repo",):
    if _p not in sys.path and os.path.isdir(_p):
        sys.path.insert(0, _p)

from contextlib import ExitStack

import ml_dtypes
import numpy as np

import concourse.bacc as bacc
import concourse.mybir as mybir
import concourse.tile as tile
from concourse import bass_utils

B, T, C, H = 4, 2048, 1024, 16
D = 64                 # head dim
HLOC = 8               # heads per core
CS = HLOC * D          # 512: per-core slice of C on the head axis
SCALE = 1.0 / 8.0      # 1/sqrt(D)
NP = 128               # partitions
QC = 512               # q chunk (PSUM bank width in fp32)
NQC = T // QC          # 4
NKT = T // NP          # 16 k tiles
NCT = C // NP          # 8 contraction tiles for projections
NPAIR = HLOC // 2      # 4 head pairs

F32 = mybir.dt.float32
BF16 = mybir.dt.bfloat16
BF = ml_dtypes.bfloat16

_CACHE = {}


def _build():
    nc = bacc.Bacc(
        "TRN2",
        target_bir_lowering=False,
        debug=False,
        enable_asserts=False,
        num_devices=8,
    )

    qT_d = nc.dram_tensor("qT", [C, T], BF16, kind="ExternalInput").ap()
    kT_d = nc.dram_tensor("kT", [C, T], BF16, kind="ExternalInput").ap()
    vT_d = nc.dram_tensor("vT", [C, T], BF16, kind="ExternalInput").ap()
    Wq_d = nc.dram_tensor("Wq", [C, CS], BF16, kind="ExternalInput").ap()
    Wk_d = nc.dram_tensor("Wk", [C, CS], BF16, kind="ExternalInput").ap()
    Wv_d = nc.dram_tensor("Wv", [C, CS], BF16, kind="ExternalInput").ap()
    Wo_d = nc.dram_tensor("Wo", [CS, C], BF16, kind="ExternalInput").ap()
    bq_d = nc.dram_tensor("bq", [CS], F32, kind="ExternalInput").ap()
    bk_d = nc.dram_tensor("bk", [CS], F32, kind="ExternalInput").ap()
    y_d = nc.dram_tensor("y", [T, C], F32, kind="ExternalOutput").ap()

    with tile.TileContext(nc) as tc, ExitStack() as ctx:
        wpool = ctx.enter_context(tc.tile_pool(name="wpool", bufs=1))
        cpool = ctx.enter_context(tc.tile_pool(name="cpool", bufs=1))
        xpool = ctx.enter_context(tc.tile_pool(name="xpool", bufs=2))
        epool = ctx.enter_context(tc.tile_pool(name="epool", bufs=1))
        spool = ctx.enter_context(tc.tile_pool(name="spool", bufs=3))
        ypool = ctx.enter_context(tc.tile_pool(name="ypool", bufs=3))
        psP = ctx.enter_context(tc.tile_pool(name="psP", bufs=2, space="PSUM"))
        psS = ctx.enter_context(tc.tile_pool(name="psS", bufs=2, space="PSUM"))
        psO = ctx.enter_context(tc.tile_pool(name="psO", bufs=2, space="PSUM"))

        # ---- persistent weights / consts ----
        # Weights live in single wide tiles. DMA priority order: Wq + qT
        # first-chunk quarters first (interleaved across the SP and ACT
        # queues) so the Q projection can start after ~256KB instead of ~1MB.
        def w_tile_and_dma(name, dram, n_ct, width, dt):
            t = wpool.tile([NP, n_ct * width], dt, name=name, tag=name)
            nc.sync.dma_start(
                t.rearrange("p (j n) -> p j n", n=width),
                dram.rearrange("(j p) n -> p j n", p=NP),
            )
            return t

        xcur = {}

        def emit_x_dma(c, inputs=(0, 1, 2)):
            for ii in inputs:
                x_d = (qT_d, kT_d, vT_d)[ii]
                xt = xpool.tile([NP, NCT * QC], BF16, name=f"x{ii}", tag=f"x{ii}")
                nc.sync.dma_start(
                    xt.rearrange("p (j n) -> p j n", n=QC),
                    x_d[:, c * QC : (c + 1) * QC].rearrange("(j p) n -> p j n", p=NP),
                )
                xcur[(c, ii)] = xt

        def w_part_dma(t, dram, width, i, parts):
            h = NCT // parts
            nc.sync.dma_start(
                t[:, i * h * width : (i + 1) * h * width].rearrange(
                    "p (j n) -> p j n", n=width
                ),
                dram[i * h * NP : (i + 1) * h * NP, :].rearrange(
                    "(j p) n -> p j n", p=NP
                ),
            )

        def x_part_dma(c, ii, i, parts):
            x_d = (qT_d, kT_d, vT_d)[ii]
            h = NCT // parts
            nc.scalar.dma_start(
                xcur[(c, ii)][:, i * h * QC : (i + 1) * h * QC].rearrange(
                    "p (j n) -> p j n", n=QC
                ),
                x_d[i * h * NP : (i + 1) * h * NP, c * QC : (c + 1) * QC]
                .rearrange("(j p) n -> p j n", p=NP),
            )

        # Attention chunks are processed in order 1, 3, 2, 0: the first chunk
        # is medium-cost (overlaps the input DMA stream), the exp-heaviest
        # chunks run mid-kernel where mandatory projection work doubles as
        # PE filler for the ACT-bound exp chains, and the kernel ends on
        # chunk 0 whose exp chains are tiny (short tail).
        CORDER = [1, 3, 2, 0]
        C0 = CORDER[0]

        Wq_sb = wpool.tile([NP, NCT * CS], BF16, name="Wq", tag="Wq")
        Wk_sb = wpool.tile([NP, NCT * CS], BF16, name="Wk", tag="Wk")
        xcur[(C0, 0)] = xpool.tile([NP, NCT * QC], BF16, name="x0", tag="x0")
        xcur[(C0, 1)] = xpool.tile([NP, NCT * QC], BF16, name="x1", tag="x1")
        bq_sb = cpool.tile([NP, 4], F32, name="bq_sb", tag="bq_sb")
        bk_sb = cpool.tile([NP, 4], F32, name="bk_sb", tag="bk_sb")
        # Wq/x in quarters, interleaved: first matmul can start after one
        # quarter of each has landed (~2 x 256KB).
        for i in range(4):
            w_part_dma(Wq_sb, Wq_d, CS, i, 4)
            x_part_dma(C0, 0, i, 4)
            if i == 0:
                nc.sync.dma_start(bq_sb[:], bq_d.rearrange("(t p) -> p t", p=NP))
        for i in range(4):
            w_part_dma(Wk_sb, Wk_d, CS, i, 4)
            x_part_dma(C0, 1, i, 4)
            if i == 0:
                nc.sync.dma_start(bk_sb[:], bk_d.rearrange("(t p) -> p t", p=NP))
        Wv_sb = w_tile_and_dma("Wv", Wv_d, NCT, CS, BF16)
        emit_x_dma(0, inputs=(2,))
        emit_x_dma(1, inputs=(2,))
        Wo_sb = w_tile_and_dma("Wo", Wo_d, NPAIR, C, BF16)

        # Causal mask for the single partially-masked [128,128] block of each
        # diagonal tile. Duplicated in two halves so both heads mask with one
        # op: mask[p, f%128] = 1.0 if f%128 >= p.
        mask_sb = cpool.tile([NP, 2 * NP], BF16, name="mask_sb", tag="mask_sb")
        nc.gpsimd.memset(mask_sb[:], 1.0)
        nc.gpsimd.affine_select(
            out=mask_sb.rearrange("p (h f) -> p h f", h=2),
            in_=mask_sb.rearrange("p (h f) -> p h f", h=2),
            pattern=[[0, 2], [1, NP]],
            compare_op=mybir.AluOpType.is_ge,
            fill=0.0,
            base=0,
            channel_multiplier=-1,
        )

        # identity (permutation) matrix for PE transposes
        ident_sb = cpool.tile([NP, NP], BF16, name="ident_sb", tag="ident_sb")
        nc.gpsimd.memset(ident_sb[:], 1.0)
        nc.gpsimd.affine_select(
            out=ident_sb[:],
            in_=ident_sb[:],
            pattern=[[1, NP]],
            compare_op=mybir.AluOpType.is_equal,
            fill=0.0,
            base=0,
            channel_multiplier=-1,
        )

        # persistent activations
        QT_sb = [
            cpool.tile([NP, T], BF16, name=f"QT{p}", tag=f"QT{p}") for p in range(NPAIR)
        ]
        KT_sb = [
            cpool.tile([NP, T], BF16, name=f"KT{p}", tag=f"KT{p}") for p in range(NPAIR)
        ]
        ON_sb = [
            cpool.tile([NP, T], BF16, name=f"ON{p}", tag=f"ON{p}") for p in range(NPAIR)
        ]
        # V' padded: per k-tile [128, 8 heads * 65], col 64 of each 65-block = 1.0
        V_sb = [
            cpool.tile([NP, HLOC * 65], BF16, name=f"V{t}", tag=f"V{t}")
            for t in range(NKT)
        ]
        for t in range(NKT):
            v3 = V_sb[t].rearrange("p (h e) -> p h e", e=65)
            nc.any.memset(v3[:, :, 64:65], 1.0)

        # ---------------- emission helpers ----------------
        def gen_qk_proj_first(W_sb, b_sb, OUT, ii, ots, qc):
            """First-chunk Q/K projection for an ot pair, j-major in quarters
            to match the quarter-DMA arrival order."""
            pss = {
                ot: psP.tile([NP, QC], F32, name="ps_proj", tag="ps_proj")
                for ot in ots
            }
            for jq in range(4):
                for ot in ots:
                    for j in (jq * 2, jq * 2 + 1):
                        last = j == NCT - 1

                        def mm(ps=pss[ot], ot=ot, j=j, last=last):
                            nc.tensor.matmul(
                                ps[:],
                                W_sb[:, j * CS + ot * NP : j * CS + (ot + 1) * NP],
                                xcur[(qc, ii)][:, j * QC : (j + 1) * QC],
                                start=(j == 0),
                                stop=last,
                            )
                            if last:
                                nc.vector.tensor_scalar_add(
                                    OUT[ot][:, qc * QC : (qc + 1) * QC],
                                    ps[:],
                                    b_sb[:, ot : ot + 1],
                                )

                        yield mm

        def gen_qk_proj_part(qc, ot):
            """Q^T and K^T projection matmuls for chunk qc, c_out tile ot.
            Chunk 0 is processed last, where ACT is idle and DVE-queue
            latency gates the next QK matmuls — evacuate via ACT there."""
            for ii, (W_sb, b_sb, OUT) in enumerate(
                ((Wq_sb, bq_sb, QT_sb), (Wk_sb, bk_sb, KT_sb))
            ):
                ps = psP.tile([NP, QC], F32, name="ps_proj", tag="ps_proj")
                for j in range(NCT):
                    last = j == NCT - 1

                    def mm(ps=ps, ii=ii, W_sb=W_sb, b_sb=b_sb, OUT=OUT, j=j, last=last):
                        nc.tensor.matmul(
                            ps[:],
                            W_sb[:, j * CS + ot * NP : j * CS + (ot + 1) * NP],
                            xcur[(qc, ii)][:, j * QC : (j + 1) * QC],
                            start=(j == 0),
                            stop=last,
                        )
                        if last:
                            nc.vector.tensor_scalar_add(
                                OUT[ot][:, qc * QC : (qc + 1) * QC],
                                ps[:],
                                b_sb[:, ot : ot + 1],
                            )

                    yield mm

        def gen_v_proj_part(qc, ts):
            """V projection matmuls for chunk qc, t-subtile ts."""
            t_tile = qc * 4 + ts
            ps = psP.tile([NP, QC], F32, name="ps_proj", tag="ps_proj")
            for j in range(NCT):
                last = j == NCT - 1

                def mm(ps=ps, j=j, last=last, t_tile=t_tile):
                    nc.tensor.matmul(
                        ps[:],
                        xcur[(qc, 2)][:, j * QC + ts * NP : j * QC + (ts + 1) * NP],
                        Wv_sb[:, j * CS : (j + 1) * CS],
                        start=(j == 0),
                        stop=last,
                    )
                    if last:
                        dst = V_sb[t_tile].rearrange("p (h e) -> p h e", e=65)[
                            :, :, 0:64
                        ]
                        src = ps.rearrange("p (h d) -> p h d", d=D)
                        nc.vector.tensor_copy(dst, src)

                yield mm

        def gen_out_proj_part(tc_, p, act_evac=False):
            """Out-projection for t_tile 4*tc_+p, both 512-wide n chunks.
            act_evac: evacuate via ACT — for groups consumed at the tail,
            where ACT is idle but the DVE queue (normalizes, ON copies)
            gates the PE."""
            tt = 4 * tc_ + p
            tsl = slice(tt * NP, (tt + 1) * NP)
            ysb = ypool.tile([NP, C], F32, name="ysb", tag="ysb")
            for nck in range(2):
                nsl = slice(nck * QC, (nck + 1) * QC)
                ps = psP.tile([NP, QC], F32, name="ps_proj", tag="ps_proj")
                for pair in range(NPAIR):
                    last = pair == NPAIR - 1

                    def mm(ps=ps, pair=pair, last=last, tsl=tsl, nsl=nsl, nck=nck):
                        nc.tensor.matmul(
                            ps[:],
                            ON_sb[pair][:, tsl],
                            Wo_sb[:, pair * C + nsl.start : pair * C + nsl.stop],
                            start=(pair == 0),
                            stop=last,
                        )
                        if last:
                            if act_evac:
                                nc.scalar.copy(ysb[:, nsl], ps[:])
                            else:
                                nc.vector.tensor_copy(ysb[:, nsl], ps[:])
                            if nck == 1:
                                nc.sync.dma_start(y_d[tsl, :], ysb[:])

                    yield mm

        def emit_outproj_tile(tt, nw=QC):
            """Inline out-projection of one t-tile in n-chunks of nw, heads
            pair 3 last (its ON slice is the freshest), DVE-side evacuation
            (ACT is exp-saturated in the late chunks), DMA per n-chunk."""
            tsl = slice(tt * NP, (tt + 1) * NP)
            ysb = ypool.tile([NP, C], F32, name="ysb", tag="ysb")
            for nck in range(C // nw):
                nsl = slice(nck * nw, (nck + 1) * nw)
                ps = psP.tile([NP, nw], F32, name="ps_proj", tag="ps_proj")
                for pair in range(NPAIR):
                    nc.tensor.matmul(
                        ps[:],
                        ON_sb[pair][:, tsl],
                        Wo_sb[:, pair * C + nsl.start : pair * C + nsl.stop],
                        start=(pair == 0),
                        stop=(pair == NPAIR - 1),
                    )
                nc.vector.tensor_copy(ysb[:, nsl], ps[:])
                # alternate DMA queues so the final transfers don't queue
                # behind earlier y DMAs on the in-order SP sequencer
                eng = nc.scalar if nck % 2 else nc.sync
                eng.dma_start(y_d[tsl, nsl], ysb[:, nsl])

        # ---------------- filler queues ----------------
        # items: (attention-order index, closure); the index caps front-running
        proj_q = []
        mark_qk = {}   # (qc, pair) -> proj_q index that must be drained first
        mark_av = {}   # qc -> proj_q index that must be drained before AV

        def x_dma_item(c, inputs):
            def f(c=c, inputs=tuple(inputs)):
                emit_x_dma(c, inputs=inputs)

            return f

        # first chunk (C0): Q(ot 0,1), K(ot 0,1) -> pairs 0,1 ready early.
        for ots in ((0, 1), (2, 3)):
            for ii, W_sb, b_sb, OUT in (
                (0, Wq_sb, bq_sb, QT_sb),
                (1, Wk_sb, bk_sb, KT_sb),
            ):
                proj_q.extend(
                    (0, f)
                    for f in gen_qk_proj_first(W_sb, b_sb, OUT, ii, ots, C0)
                )
            mark_qk[(C0, ots[0])] = mark_qk[(C0, ots[1])] = len(proj_q)
        # V projections for chunks 0,1 (the first attention chunk needs both);
        # per-tile marks let the AV steps drain V just-in-time
        mark_v = {}
        for c in (0, 1):
            for ts in range(NPAIR):
                proj_q.extend((0, f) for f in gen_v_proj_part(c, ts))
                mark_v[4 * c + ts] = len(proj_q)

        # chunk 3 Q/K (attention order 1), with V for chunks 2,3 inside
        proj_q.append((1, x_dma_item(3, (0, 1))))
        for p in range(NPAIR):
            proj_q.extend((1, f) for f in gen_qk_proj_part(3, p))
            mark_qk[(3, p)] = len(proj_q)
            if p == 0:
                proj_q.append((1, x_dma_item(2, (2,))))
                for ts in range(NPAIR):
                    proj_q.extend((1, f) for f in gen_v_proj_part(2, ts))
                    mark_v[8 + ts] = len(proj_q)
                proj_q.append((1, x_dma_item(3, (2,))))
                for ts in range(NPAIR):
                    proj_q.extend((1, f) for f in gen_v_proj_part(3, ts))
                    mark_v[12 + ts] = len(proj_q)
        # chunk 2 Q/K (attention order 2)
        proj_q.append((2, x_dma_item(2, (0, 1))))
        for p in range(NPAIR):
            proj_q.extend((2, f) for f in gen_qk_proj_part(2, p))
            mark_qk[(2, p)] = len(proj_q)
        # chunk 0 Q/K (attention order 3)
        proj_q.append((3, x_dma_item(0, (0, 1))))
        for p in range(NPAIR):
            proj_q.extend((3, f) for f in gen_qk_proj_part(0, p))
            mark_qk[(0, p)] = len(proj_q)

        op_q = []      # eligible out-proj closures (appended as chunks finish)

        state = {"pq": 0, "qc": 0}

        def drain_to(idx):
            while state["pq"] < idx:
                proj_q[state["pq"]][1]()
                state["pq"] += 1

        def splice(n):
            # pop projection filler, but never front-run more than one chunk
            # ahead of the current attention chunk
            k = 0
            while (
                k < n
                and state["pq"] < len(proj_q)
                and proj_q[state["pq"]][0] <= state["qc"] + 1
            ):
                proj_q[state["pq"]][1]()
                state["pq"] += 1
                k += 1
            if k == 0 and op_q:
                # ration out-proj filler so it lasts through the final chunk
                state["tick"] = state.get("tick", 0) + 1
                if state["tick"] % 2 == 0 or state["qc"] >= NQC - 2:
                    op_q.pop(0)()

        # ---------------- attention with interleaved filler ----------------
        for oi, qc in enumerate(CORDER):
            state["qc"] = oi
            kmax = 4 * (qc + 1)
            for pair in range(NPAIR):
                last_sec = oi == NQC - 1 and pair == NPAIR - 1
                drain_to(mark_qk[(qc, pair)])
                if oi == NQC - 1:
                    # the final chunk's units are short: force the excess
                    # out-proj backlog through now so none trails the last
                    # tile (and so it covers the proj bias-add latency that
                    # gates this unit's first QK matmuls)
                    while len(op_q) > (NPAIR - 1 - pair) * 12 + 8:
                        op_q.pop(0)()
                es = []
                # AV accumulators: two q-subtiles share one PSUM bank
                # (2 heads x 65 cols each), so all four qsub accumulations
                # are live at once and the AV matmuls can interleave with
                # the QK/exp pipeline k-tile-major.
                banks = [
                    psO.tile([NP, 260], F32, name="O2a", tag="O"),
                    psO.tile([NP, 260], F32, name="O2b", tag="O"),
                ]
                pairTs = []

                def norm(qs, banks=banks, pairTs=pairTs):
                    O3 = banks[qs // 2][
                        :, (qs % 2) * 130 : (qs % 2) * 130 + 130
                    ].rearrange("p (h e) -> p h e", e=65)
                    zinv = spool.tile([NP, 2], F32, name="zinv", tag="zinv")
                    nc.vector.reciprocal(zinv[:], O3[:, :, 64:65])
                    pairT = spool.tile(
                        [NP, NP], BF16, name="pairT", tag="pairT", bufs=6
                    )
                    nc.vector.tensor_mul(
                        pairT.rearrange("p (a d) -> p a d", d=64),
                        O3[:, :, 0:64],
                        zinv[:]
                        .rearrange("p (a o) -> p a o", o=1)
                        .to_broadcast((NP, 2, 64)),
                    )
                    pairTs.append(pairT)

                def tr_copy(qs, pair=pair, qc=qc, pairTs=pairTs):
                    trp = psO.tile([NP, NP], BF16, name="trp", tag="O")
                    nc.tensor.transpose(trp[:], pairTs[qs][:], ident_sb[:])
                    tt = 4 * qc + qs
                    nc.vector.tensor_copy(
                        ON_sb[pair][:, tt * NP : (tt + 1) * NP], trp[:]
                    )

                # Merged QK/exp/AV pipeline, k-tile-major with a 2-tile lag:
                # each step runs QK(kt) and the AV matmuls of kt-LAG across
                # all valid q-subtiles, keeping the PE naturally paced just
                # under the ACT exp chain (~930ns/tile) with only ~1 filler
                # matmul of slack per tile. es[kt] is fully consumed at step
                # kt+LAG, releasing it early for the next unit's exp.
                LAG = 4
                for step in range(kmax + LAG):
                    if step < kmax:
                        kt = step
                        # diagonal tiles (kt >= 4*qc) only need the q-suffix
                        # [off, 512): columns below are fully causal-masked
                        off = max(0, (kt - 4 * qc) * NP)
                        ksl = slice(kt * NP, (kt + 1) * NP)
                        S2 = psS.tile([NP, 2 * QC], F32, name="S2", tag="S2")
                        for hp in range(2):
                            psl = slice(hp * 64, (hp + 1) * 64)
                            nc.tensor.matmul(
                                S2[:, hp * QC + off : (hp + 1) * QC],
                                KT_sb[pair][psl, ksl],
                                QT_sb[pair][psl, qc * QC + off : (qc + 1) * QC],
                                start=True,
                                stop=True,
                                tile_position=(hp * 64, 0),
                            )
                        e2 = epool.tile(
                            [NP, 2 * QC], BF16, name=f"e{kt}", tag=f"e{kt}"
                        )
                        s3 = S2.rearrange("p (h f) -> p h f", h=2)[:, :, off:]
                        e3 = e2.rearrange("p (h f) -> p h f", h=2)[:, :, off:]
                        nc.scalar.activation(
                            e3, s3, mybir.ActivationFunctionType.Exp, scale=SCALE
                        )
                        if off or kt == 4 * qc:  # diagonal: mask partial block
                            eb = e2.rearrange("p (h f) -> p h f", h=2)[
                                :, :, off : off + NP
                            ]
                            nc.vector.tensor_mul(
                                eb, eb, mask_sb.rearrange("p (h f) -> p h f", h=2)
                            )
                        es.append(e2)
                        # deferred transposes+copies of the previous unit's
                        # subtiles: the last normalize's latency is covered
                        # by this unit's early QK phase
                        if kt == 3 and state.get("pending") is not None:
                            state["pending"]()
                            state["pending"] = None
                    kt2 = step - LAG
                    if 0 <= kt2 < kmax:
                        drain_to(mark_v[kt2])
                        for qs in range(max(0, kt2 - 4 * qc), 4):
                            bank = banks[qs // 2]
                            base = (qs % 2) * 130
                            for hp in range(2):
                                h = pair * 2 + hp
                                nc.tensor.matmul(
                                    bank[
                                        :, base + hp * 65 : base + (hp + 1) * 65
                                    ],
                                    es[kt2][
                                        :,
                                        hp * QC + qs * NP : hp * QC
                                        + (qs + 1) * NP,
                                    ],
                                    V_sb[kt2][:, h * 65 : (h + 1) * 65],
                                    start=(
                                        kt2 == 0 and hp == 0 and qs % 2 == 0
                                    ),
                                    stop=(
                                        hp == 1
                                        and qs % 2 == 1
                                        and kt2 == 4 * qc + qs
                                    ),
                                )
                        # a q-subtile's chain completes at its diagonal tile
                        if kt2 >= 4 * qc:
                            norm(kt2 - 4 * qc)
                    # early steps carry no AV matmuls yet: room for one more
                    splice(2 if step < LAG + 3 else 1)
                if last_sec:
                    # transposes inline, interleaved with the final tiles'
                    # out-projections (which double as normalize-latency
                    # cover); tile 12+qs is ready right after tr_copy(qs)
                    for qs in range(4):
                        tr_copy(qs)
                        emit_outproj_tile(
                            4 * qc + qs, nw=QC if qs < 3 else QC // 2
                        )
                else:
                    # all four transposes deferred into the next unit: the
                    # psO ring then reuses each bank only after both its
                    # subtiles' normalizes have long completed
                    def pending(pair=pair, qc=qc, tr_copy=tr_copy):
                        for qs in range(4):
                            tr_copy(qs)
                        if pair == NPAIR - 1:
                            # chunk complete: its out-proj becomes eligible.
                            # Chunk 2's groups are consumed at the tail where
                            # ACT is idle and the DVE queue gates the PE.
                            for p in range(NPAIR):
                                op_q.extend(
                                    gen_out_proj_part(qc, p, act_evac=qc == 2)
                                )

                    state["pending"] = pending

        # ---------------- epilogue (normally everything is drained) --------
        drain_to(len(proj_q))
        while op_q:
            op_q.pop(0)()

    nc.compile()
    return nc


def get_nc():
    if "nc" not in _CACHE:
        _CACHE["nc"] = _build()
    return _CACHE["nc"]


def make_in_maps(k, v, q, Wq, bq, Wk, bk, Wv, bv, Wo, bo):
    k = np.asarray(k, np.float32)
    v = np.asarray(v, np.float32)
    q = np.asarray(q, np.float32)
    Wq = np.asarray(Wq, np.float32).astype(BF)
    Wk = np.asarray(Wk, np.float32).astype(BF)
    Wv = np.asarray(Wv, np.float32).astype(BF)
    Wo = np.asarray(Wo, np.float32).astype(BF)
    bq = np.asarray(bq, np.float32)
    bk = np.asarray(bk, np.float32)

    in_maps = []
    for core in range(8):
        b, hh = core // 2, core % 2
        sl = slice(hh * CS, (hh + 1) * CS)
        in_maps.append(
            {
                "qT": np.ascontiguousarray(q[b].T.astype(BF)),
                "kT": np.ascontiguousarray(k[b].T.astype(BF)),
                "vT": np.ascontiguousarray(v[b].T.astype(BF)),
                "Wq": np.ascontiguousarray(Wq[:, sl]),
                "Wk": np.ascontiguousarray(Wk[:, sl]),
                "Wv": np.ascontiguousarray(Wv[:, sl]),
                "Wo": np.ascontiguousarray(Wo[sl, :]),
                "bq": np.ascontiguousarray(bq[sl]),
                "bk": np.ascontiguousarray(bk[sl]),
            }
        )
    return in_maps


def kernel(k, v, q, Wq, bq, Wk, bk, Wv, bv, Wo, bo):
    nc = get_nc()
    in_maps = make_in_maps(k, v, q, Wq, bq, Wk, bk, Wv, bv, Wo, bo)
    res = bass_utils.run_bass_kernel_spmd(nc, in_maps, core_ids=list(range(8)))
    # softmax rows sum to 1, so the V bias passes through attention as a
    # constant: y += bv @ Wo. Fold it into the host-side bias add.
    bias = np.asarray(bo, np.float32) + np.asarray(bv, np.float32) @ np.asarray(
        Wo, np.float32
    )
    out = np.empty((B, T, C), np.float32)
    for b in range(B):
        out[b] = res.results[2 * b]["y"] + res.results[2 * b + 1]["y"] + bias
    return out
